# revision 39
# baseline (speedup 1.0000x reference)
"""MoE BERT head (soft routing) on 8 Trainium2 NeuronCores.

Reference computation (B=4096, H=1024, E=32, O=1024):
    gate = softmax(X @ gate_W + gate_b, axis=experts)            [B, E]
    h_e  = relu(LN(X @ W1[e] + b1[e]) * ln_g[e] + ln_b[e])       [B, H] per expert
    out  = sum_e gate[:, e] * (h_e @ W2[e] + b2[e])              [B, O]

Strategy: expert-parallel over 8 cores (4 experts/core), two passes of 2
experts per core, batch in column-chunks of 2048.  All matmul operands are
bf16 (measured end-to-end max-rel error 2.8e-3 vs the 2e-2 gate); on this
HW bf16 matmuls run slightly faster than fp32r and halve DMA/SBUF.  Per
chunk each expert's first GEMM runs in [batch, feature] layout processing
batch tiles in PAIRS with a 4-way PSUM-bank rotation (measured faster than
2-bank alternation or bank-sequential ordering).  LayerNorm stats come from
bn_stats directly on PSUM and the normalization (with the softmax gating
weight folded in as a per-row scale) is applied by a fused DVE
tensor_scalar.  The PE's GEMM1 PSUM banks are drained by ACT copies into a
bf16 SBUF staging tile and bn_stats/apply read THAT, not PSUM: concurrent
DVE PSUM reads measurably slow PE writes (~75 us), and bf16 SBUF reads run
at 2x DVE rate.  Activations are
PE-transposed (bf16 transposes; grouped 8 per PSUM tile) into
[feature, batch] layout with relu applied on the PSUM->SBUF copyback,
alternating ACT/DVE (either engine alone cannot keep up with the PE).  The
second GEMM accumulates BOTH experts in PSUM (K-concatenation, K=2048),
two bs-groups interleaved, psum shared with the GEMM1 pool (6 slots + 2
transpose banks = 8 banks).  X and W1 live in single mega-tiles sliced per
matmul.  Each core returns per-pass partial outputs out.T [O, B] in bf16
(halves the out-DMA; partials summed in f32 on the host); the per-expert
output bias enters the total as gate @ b2, added on the host (exact).

Measured HW notes (microbenchmarks, this device): a 512-row bf16 matmul
costs ~240-280 ns (not the 213 ns cost-model ideal) because the stationary
load does not fully overlap; [128,128] transposes ~170 ns grouped.  fp8
e4m3 DoubleRow gives ~1.67x GEMM throughput but plain fp8 quantization
measures 2.2e-2 (k-split half-fp8 1.7e-2, expert-split 1.9e-2) against
the 2e-2 error gate, and mixing DoubleRow with bf16 instructions inside
one accumulation group costs ~PE mode switches that erase the gain -- the
fp8 path is kept behind _CFG["fp8_kp"] but disabled.
"""

import os
import sys
from contextlib import ExitStack

for _p in ("/opt/trn_rl_repo", "/root/.axon_site/_ro/trn_rl_repo"):
    if os.path.isdir(_p) and _p not in sys.path:
        sys.path.insert(0, _p)

import numpy as np

import concourse.bass as bass
import concourse.mybir as mybir
import concourse.tile as tile
from concourse import bacc
from concourse.bass_utils import run_bass_kernel_spmd
from concourse.masks import make_identity

B, H, E, O = 4096, 1024, 32, 1024
LN_EPS = 1e-5
N_CORES = 8
E_PER_CORE = E // N_CORES            # 4
N_PASSES = 2
E_PER_PASS = E_PER_CORE // N_PASSES  # 2
B_CHUNK = 2048
N_CHUNKS = B // B_CHUNK              # 2
P = 128
KT = H // P                          # 8 k-tiles over the hidden dim
BT = B_CHUNK // P                    # 16 batch tiles per chunk
F32 = mybir.dt.float32
BF16 = mybir.dt.bfloat16
FP8 = mybir.dt.float8e4
DR = mybir.MatmulPerfMode.DoubleRow
NKP8 = 2        # k-pairs (256 features each) of GEMM1 contraction done in fp8
KF8 = NKP8 * 2 * P   # 512 features
KTB = KT - NKP8 * 2  # remaining bf16 k-tiles (4)

Relu = mybir.ActivationFunctionType.Relu
Exp = mybir.ActivationFunctionType.Exp
Sqrt = mybir.ActivationFunctionType.Sqrt
Alu = mybir.AluOpType
AxX = mybir.AxisListType.X

_CACHE = {}
_LAST_IN_MAPS = None

# scheduling knobs (pool depths / grouping)
_CFG = {
    "xtp": 1, "w1p": 2, "w2p": 2, "hscp": 18, "hstp": 2, "osbp": 3,
    "hps": 6, "tps": 2,
    "cb_alt": 1,      # alternate transpose copybacks between ACT and DVE
    "skip_transpose": 0,   # timing-attribution only (breaks correctness)
    "skip_gemm2": 0,       # timing-attribution only (breaks correctness)
    "skip_ln": 0,          # timing-attribution only (breaks correctness)
    "fp8_kp": 0,           # GEMM1 k-pairs (256 feat each) in fp8 DoubleRow
    "g1_pair": 1,          # GEMM1: bt-pairs with 4-way PSUM bank rotation
    "t_spread": 0,         # 0 = half-major transpose order (measured best)
}


class _K:
    """Holds program-build state (pools, dram handles, flags)."""


def _load_chunk_xt(k_, c0):
    """One mega-tile [P, KT, B_CHUNK] per chunk; k-tiles are slices."""
    nc = k_.nc
    xm = k_.xtp.tile([P, KT, B_CHUNK], BF16, tag="xt", name="xm")
    for k in range(KT):
        nc.sync.dma_start(out=xm[:, k, :],
                          in_=k_.xt_d[k * P:(k + 1) * P, c0:c0 + B_CHUNK])
    x8m = None
    if _CFG["fp8_kp"]:
        x8m = k_.xtp.tile([P, NKP8, 2, B_CHUNK], FP8, tag="xt8", name="x8m")
        nc.sync.dma_start(out=x8m[:], in_=k_.xt8_d[:, :, :, c0:c0 + B_CHUNK])
    return [xm[:, k, :] for k in range(KT)], x8m


def _gate_softmax(k_, xts):
    """gate = softmax(X @ gate_W + gate_b) for all E; returns gsb [P, BT, E]."""
    nc = k_.nc
    g_ps = k_.hps.tile([P, BT, E], F32, tag="hps", name="g_ps")
    for bt in range(BT):
        for k in range(KT):
            nc.tensor.matmul(
                g_ps[:, bt, :],
                xts[k][:, bt * P:(bt + 1) * P],
                k_.gw_sb[:, k, :],
                start=(k == 0), stop=(k == KT - 1))
    gsb = k_.gselp.tile([P, BT, E], F32, tag="gsb")
    nc.scalar.copy(gsb[:], g_ps[:])
    if k_.use_gb:
        for bt in range(BT):
            nc.vector.tensor_add(gsb[:, bt, :], gsb[:, bt, :], k_.gb_bc[:])
    gmax = k_.smallp.tile([P, BT], F32, tag="gmax")
    nc.vector.tensor_reduce(gmax[:], gsb[:], axis=AxX, op=Alu.max)
    nc.vector.tensor_scalar_mul(gmax[:], gmax[:], -1.0)
    for bt in range(BT):
        nc.scalar.activation(gsb[:, bt, :], gsb[:, bt, :], Exp,
                             bias=gmax[:, bt:bt + 1])
    gsum = k_.smallp.tile([P, BT], F32, tag="gsum")
    nc.vector.tensor_reduce(gsum[:], gsb[:], axis=AxX, op=Alu.add)
    nc.vector.reciprocal(gsum[:], gsum[:])
    for bt in range(BT):
        nc.vector.tensor_scalar_mul(gsb[:, bt, :], gsb[:, bt, :],
                                    gsum[:, bt:bt + 1])
    return gsb



def _ln_apply(k_, ps, gsb, e, bt, affine, b1_bc, lng_bc, lnb_bc):
    """LayerNorm stats + fused apply for one batch tile; returns hbf bf16."""
    nc = k_.nc
    stats = k_.smallp.tile([P, 2, 6], F32, tag="stats")
    h32 = None
    if affine:
        h32 = k_.hscp32.tile([P, H], F32, tag="h32")
        for d in range(2):
            sl = slice(d * 512, (d + 1) * 512)
            if k_.use_b1:
                nc.vector.tensor_add(h32[:, sl], ps[d][:], b1_bc[:, sl])
            else:
                nc.scalar.copy(h32[:, sl], ps[d][:])
            nc.vector.bn_stats(stats[:, d, :], h32[:, sl])
    else:
        # ACT copies PSUM->SBUF bf16 once; stats+apply then run from SBUF
        # (DVE reading PSUM while the PE writes other banks measurably slows
        # the PE; bf16 SBUF reads also run at 2x DVE rate)
        hb_pre = k_.hprep.tile([P, H], BF16, tag="hpre")
        for d in range(2):
            sl = slice(d * 512, (d + 1) * 512)
            nc.scalar.copy(hb_pre[:, sl], ps[d][:])
            nc.vector.bn_stats(stats[:, d, :], hb_pre[:, sl])
    mv = k_.smallp.tile([P, 2], F32, tag="mv")
    nc.vector.bn_aggr(mv[:], stats[:])
    rg = k_.smallp.tile([P, 1], F32, tag="rg")
    nc.scalar.activation(rg[:], mv[:, 1:2], Sqrt, bias=k_.eps_t[:])
    nc.vector.reciprocal(rg[:], rg[:])
    nc.vector.tensor_mul(rg[:], rg[:], gsb[:, bt, e:e + 1])
    hbf = k_.hscp.tile([P, H], BF16, tag="hsc")
    if affine:
        nc.vector.tensor_scalar(
            h32[:], h32[:], mv[:, 0:1], rg[:],
            op0=Alu.subtract, op1=Alu.mult)
        if k_.use_lng:
            nc.vector.tensor_mul(h32[:], h32[:], lng_bc[:])
        if k_.use_lnb:
            # h += ln_b * gate   (gate>0 commutes with the later relu)
            nc.vector.scalar_tensor_tensor(
                h32[:], lnb_bc[:], gsb[:, bt, e:e + 1], h32[:],
                op0=Alu.mult, op1=Alu.add)
        nc.vector.tensor_copy(hbf[:], h32[:])
    else:
        nc.vector.tensor_scalar(
            hbf[:], hb_pre[:], mv[:, 0:1], rg[:],
            op0=Alu.subtract, op1=Alu.mult)
    return hbf



def _transpose_half(k_, hbfs, hsT_e, half, ks, cbc):
    """Transpose+relu copyback for one batch half (bt half*8..half*8+7)."""
    nc = k_.nc
    for k in ks:
        tp = k_.tps.tile([P, 1024], BF16, tag="tps")
        for q in range(8):
            bt = half * 8 + q
            nc.tensor.transpose(
                tp[:, q * P:(q + 1) * P],
                hbfs[bt][:, k * P:(k + 1) * P],
                k_.ident[:])
        cb_dst = hsT_e[:, k, half * 1024:(half + 1) * 1024]
        if _CFG["cb_alt"] == 2 or (_CFG["cb_alt"] == 1 and cbc[0] % 2 == 1):
            nc.vector.tensor_scalar_max(cb_dst, tp[:], 0.0)
        else:
            nc.scalar.activation(cb_dst, tp[:], Relu)
        cbc[0] += 1


def _transpose_relu(k_, hbfs):
    """PE transpose -> relu -> hsT [P, KT, B_CHUNK] bf16 [feature, batch]."""
    nc = k_.nc
    hsT_e = k_.hstp.tile([P, KT, B_CHUNK], BF16, tag="hsT")
    if _CFG["skip_transpose"]:
        nc.vector.memset(hsT_e[:, :, 0:4], 0.0)
        return hsT_e
    cb = 0
    for k in range(KT):
        for half in range(BT // 8):
            tp = k_.tps.tile([P, 1024], BF16, tag="tps")
            for q in range(8):
                bt = half * 8 + q
                nc.tensor.transpose(
                    tp[:, q * P:(q + 1) * P],
                    hbfs[bt][:, k * P:(k + 1) * P],
                    k_.ident[:])
            cb_dst = hsT_e[:, k, half * 1024:(half + 1) * 1024]
            if _CFG["cb_alt"] == 2 or (_CFG["cb_alt"] == 1 and cb % 2 == 1):
                nc.vector.tensor_scalar_max(cb_dst, tp[:], 0.0)
            else:
                nc.scalar.activation(cb_dst, tp[:], Relu)
            cb += 1
    return hsT_e


def _expert_gemm1_ln(k_, xts, x8m, gsb, e):
    """GEMM1 + LayerNorm + gating fold + PE transpose for local expert e.

    Returns hsT_e [P, KT, B_CHUNK] bf16 in [feature, batch] layout,
    already relu'd and scaled by the gating weight.
    """
    nc = k_.nc
    b1_bc = lng_bc = lnb_bc = None
    if k_.use_b1:
        b1_bc = k_.bcp.tile([P, H], F32, tag="b1bc")
        nc.gpsimd.dma_start(out=b1_bc[:], in_=k_.b1_d[e].partition_broadcast(P))
    if k_.use_lng:
        lng_bc = k_.bcp.tile([P, H], F32, tag="lngbc")
        nc.gpsimd.dma_start(out=lng_bc[:], in_=k_.lng_d[e].partition_broadcast(P))
    if k_.use_lnb:
        lnb_bc = k_.bcp.tile([P, H], F32, tag="lnbbc")
        nc.gpsimd.dma_start(out=lnb_bc[:], in_=k_.lnb_d[e].partition_broadcast(P))
    affine = k_.use_b1 or k_.use_lng or k_.use_lnb

    nkp = _CFG["fp8_kp"]
    ktb0 = nkp * 2                      # first bf16 k-tile
    w18m = None
    if nkp:
        w18m = k_.w1p.tile([P, NKP8, 2, H], FP8, tag="w18", name="w18m")
        nc.sync.dma_start(out=w18m[:], in_=k_.w18_d[e])
        w1m = k_.w1p.tile([P, KT - ktb0, H], BF16, tag="w1", name="w1m")
        for i in range(KT - ktb0):
            nc.sync.dma_start(out=w1m[:, i, :], in_=k_.w1_d[e, ktb0 + i])
        w1s = {ktb0 + i: w1m[:, i, :] for i in range(KT - ktb0)}
    else:
        w1m = k_.w1p.tile([P, KT, H], BF16, tag="w1", name="w1m")
        for k in range(KT):
            nc.sync.dma_start(out=w1m[:, k, :], in_=k_.w1_d[e, k])
        w1s = {k: w1m[:, k, :] for k in range(KT)}

    hbfs = []
    if _CFG["g1_pair"]:
        # bt-pairs: 4-way PSUM bank rotation, stationary X block reused 2x.
        # The first batch-half's transposes are interleaved between the
        # second half's GEMM1 quads so their PSUM-read copybacks overlap
        # matmul stretches instead of clustering against the PE transposes.
        hsT_e = k_.hstp.tile([P, KT, B_CHUNK], BF16, tag="hsT")
        cbc = [0]
        for btp in range(BT // 2):
            pss = [k_.hps.tile([P, 512], F32, tag="hps", name=f"ps{i}")
                   for i in range(4)]
            for k in range(KT):
                for i in range(4):
                    bt = btp * 2 + i // 2
                    nc.tensor.matmul(
                        pss[i][:], xts[k][:, bt * P:(bt + 1) * P],
                        w1s[k][:, (i % 2) * 512:(i % 2 + 1) * 512],
                        start=(k == 0), stop=(k == KT - 1))
            for half in range(2):
                hbfs.append(_ln_apply(
                    k_, pss[2 * half:2 * half + 2], gsb, e, btp * 2 + half,
                    affine, b1_bc, lng_bc, lnb_bc))
            if _CFG["t_spread"] and btp >= BT // 4:
                kk = 2 * (btp - BT // 4)
                _transpose_half(k_, hbfs, hsT_e, 0, (kk, kk + 1), cbc)
        if _CFG["t_spread"]:
            _transpose_half(k_, hbfs, hsT_e, 1, range(KT), cbc)
        else:
            for half in range(2):
                _transpose_half(k_, hbfs, hsT_e, half, range(KT), cbc)
        return hsT_e

    for bt in range(BT):
        # ---- GEMM1 for this batch tile: h[bt] = X[bt] @ W1[e]  (PSUM f32)
        # fp8 k-pairs first (DoubleRow, 2 k-tiles per instruction), then bf16
        ps = [k_.hps.tile([P, 512], F32, tag="hps", name=f"ps{d}")
              for d in range(2)]
        for kp in range(nkp):
            for d in range(2):
                nc.tensor.matmul(
                    ps[d][:], x8m[:, kp, :, bt * P:(bt + 1) * P],
                    w18m[:, kp, :, d * 512:(d + 1) * 512],
                    start=(kp == 0), stop=False, perf_mode=DR)
        for k in range(ktb0, KT):
            for d in range(2):
                nc.tensor.matmul(
                    ps[d][:], xts[k][:, bt * P:(bt + 1) * P],
                    w1s[k][:, d * 512:(d + 1) * 512],
                    start=(k == 0 and nkp == 0), stop=(k == KT - 1))

        if _CFG["skip_ln"]:
            hbf = k_.hscp.tile([P, H], BF16, tag="hsc")
            for d in range(2):
                nc.scalar.copy(hbf[:, d * 512:(d + 1) * 512], ps[d][:])
            hbfs.append(hbf)
            continue

        hbfs.append(_ln_apply(k_, ps, gsb, e, bt,
                              affine, b1_bc, lng_bc, lnb_bc))

    return _transpose_relu(k_, hbfs)


def _gemm2(k_, hsT, p_i, c0):
    """out.T[p_i] += sum over both experts: W2[e].T @ hsT[e] (PSUM K-concat)."""
    nc = k_.nc
    nke = E_PER_PASS * KT
    if _CFG["skip_gemm2"]:
        osb = k_.osbp.tile([P, 1024], BF16, tag="osb")
        nc.vector.tensor_copy(osb[:, 0:4], hsT[0][:, 0, 0:4])
        nc.vector.tensor_copy(osb[:, 4:8], hsT[1][:, 0, 0:4])
        nc.sync.dma_start(
            out=k_.outp_d[p_i, 0:P, c0:c0 + 1024], in_=osb[:])
        return
    for ot in range(O // P):
        w2sb = k_.w2p.tile([P, nke, P], BF16, tag="w2")
        nc.sync.dma_start(out=w2sb[:], in_=k_.w2_d[p_i, ot])
        for bsp in range(B_CHUNK // 1024):
            osb = k_.osbp.tile([P, 1024], BF16, tag="osb")
            ops = [k_.hps.tile([P, 512], F32, tag="hps", name=f"op{i}")
                   for i in range(2)]
            for ke in range(nke):
                for half in range(2):
                    bs = bsp * 2 + half
                    nc.tensor.matmul(
                        ops[half][:],
                        w2sb[:, ke, :],
                        hsT[ke // KT][:, ke % KT, bs * 512:(bs + 1) * 512],
                        start=(ke == 0), stop=(ke == nke - 1))
            for half in range(2):
                nc.scalar.copy(osb[:, half * 512:(half + 1) * 512],
                               ops[half][:])
            nc.gpsimd.dma_start(
                out=k_.outp_d[p_i, ot * P:(ot + 1) * P,
                              c0 + bsp * 1024:c0 + (bsp + 1) * 1024],
                in_=osb[:])


def _build_program(use_gb, use_b1, use_lng, use_lnb):
    nc = bacc.Bacc("TRN2", target_bir_lowering=False, debug=False,
                   num_devices=N_CORES)
    k_ = _K()
    k_.nc = nc
    k_.use_gb, k_.use_b1, k_.use_lng, k_.use_lnb = use_gb, use_b1, use_lng, use_lnb
    affine = use_b1 or use_lng or use_lnb

    k_.xt_d = nc.dram_tensor("xt", [H, B], BF16, kind="ExternalInput")
    if _CFG["fp8_kp"]:
        k_.xt8_d = nc.dram_tensor("xt8", [P, NKP8, 2, B], FP8,
                                  kind="ExternalInput")
        k_.w18_d = nc.dram_tensor("w18", [E_PER_CORE, P, NKP8, 2, H], FP8,
                                  kind="ExternalInput")
    k_.w1_d = nc.dram_tensor("w1", [E_PER_CORE, KT, P, H], BF16,
                             kind="ExternalInput")
    k_.w2_d = nc.dram_tensor("w2t", [N_PASSES, O // P, P, E_PER_PASS * KT, P],
                             BF16, kind="ExternalInput")
    k_.gw_d = nc.dram_tensor("gw", [P, KT, E], BF16, kind="ExternalInput")
    k_.gb_d = nc.dram_tensor("gb", [E], F32, kind="ExternalInput") if use_gb else None
    k_.b1_d = (nc.dram_tensor("b1", [E_PER_CORE, H], F32, kind="ExternalInput")
               if use_b1 else None)
    k_.lng_d = (nc.dram_tensor("lng", [E_PER_CORE, H], F32, kind="ExternalInput")
                if use_lng else None)
    k_.lnb_d = (nc.dram_tensor("lnb", [E_PER_CORE, H], F32, kind="ExternalInput")
                if use_lnb else None)
    k_.outp_d = nc.dram_tensor("outp", [N_PASSES, O, B], BF16,
                               kind="ExternalOutput")

    with tile.TileContext(nc) as tc, ExitStack() as ctx:
        pool = lambda name, bufs, **kw: ctx.enter_context(
            tc.tile_pool(name=name, bufs=bufs, **kw))
        singles = pool("singles", 1)
        k_.xtp = pool("xtp", _CFG["xtp"])
        k_.w1p = pool("w1p", 1 if affine else _CFG["w1p"])
        k_.w2p = pool("w2p", _CFG["w2p"])
        k_.hscp = pool("hscp", _CFG["hscp"])
        k_.hprep = pool("hprep", 6)
        k_.hstp = pool("hstp", _CFG["hstp"])
        k_.osbp = pool("osbp", _CFG["osbp"])
        k_.smallp = pool("smallp", 20)
        k_.gselp = pool("gselp", 2)
        if affine:
            k_.bcp = pool("bcp", 1)
            k_.hscp32 = pool("hscp32", 2)
        k_.hps = pool("hps", _CFG["hps"], space="PSUM")
        k_.tps = pool("tps", _CFG["tps"], space="PSUM")

        ident_f32 = singles.tile([P, P], F32)
        make_identity(nc, ident_f32)
        k_.ident = singles.tile([P, P], BF16)
        nc.vector.tensor_copy(k_.ident[:], ident_f32[:])
        k_.eps_t = singles.tile([P, 1], F32)
        nc.vector.memset(k_.eps_t, LN_EPS)
        k_.gw_sb = singles.tile([P, KT, E], BF16)
        nc.sync.dma_start(out=k_.gw_sb[:], in_=k_.gw_d[:])
        if use_gb:
            k_.gb_bc = singles.tile([P, E], F32)
            nc.gpsimd.dma_start(out=k_.gb_bc[:],
                                in_=k_.gb_d[:].partition_broadcast(P))

        for ci in range(N_CHUNKS):
            c0 = ci * B_CHUNK
            xts, x8m = _load_chunk_xt(k_, c0)
            gsb = _gate_softmax(k_, xts)
            for p_i in range(N_PASSES):
                hsT = []
                for e01 in range(E_PER_PASS):
                    e = E_PER_PASS * p_i + e01  # local expert idx, pass-major
                    hsT.append(_expert_gemm1_ln(k_, xts, x8m, gsb, e))
                _gemm2(k_, hsT, p_i, c0)

    nc.compile()
    return nc


def kernel(pooled_output, gate_W, gate_b, W1, b1, ln_g, ln_b, W2, b2):
    import ml_dtypes
    bf16 = ml_dtypes.bfloat16

    X = np.asarray(pooled_output, dtype=np.float32)
    gate_W = np.asarray(gate_W, dtype=np.float32)
    gate_b = np.asarray(gate_b, dtype=np.float32)
    W1 = np.asarray(W1, dtype=np.float32)
    b1 = np.asarray(b1, dtype=np.float32)
    ln_g = np.asarray(ln_g, dtype=np.float32)
    ln_b = np.asarray(ln_b, dtype=np.float32)
    W2 = np.asarray(W2, dtype=np.float32)
    b2 = np.asarray(b2, dtype=np.float32)

    use_gb = bool(np.any(gate_b != 0.0))
    use_b1 = bool(np.any(b1 != 0.0))
    use_lng = bool(np.any(ln_g != 1.0))
    use_lnb = bool(np.any(ln_b != 0.0))

    key = (use_gb, use_b1, use_lng, use_lnb)
    if key not in _CACHE:
        _CACHE[key] = _build_program(*key)
    nc = _CACHE[key]

    XT = np.ascontiguousarray(X.T).astype(bf16)  # [H, B]
    XT8 = None
    if _CFG["fp8_kp"]:
        e4 = ml_dtypes.float8_e4m3
        # [P, NKP8, 2, B]: feature f = kp*256 + s*128 + p
        XT8 = np.ascontiguousarray(
            X.T[:KF8].reshape(NKP8, 2, P, B).transpose(2, 0, 1, 3)).astype(e4)

    in_maps = []
    for c in range(N_CORES):
        own = list(range(E_PER_CORE * c, E_PER_CORE * (c + 1)))
        rest = [e for e in range(E) if e not in own]
        perm = own + rest
        # W1 tiled as [e, k, 128, H]
        w1_c = np.ascontiguousarray(
            W1[own].reshape(E_PER_CORE, KT, P, H)).astype(bf16)
        # W2 tiled as [pass, o_tile, 128, (e01, kd), 128]
        w2_c = W2[own].reshape(N_PASSES, E_PER_PASS, KT, P, O // P, P)
        w2_c = np.ascontiguousarray(w2_c.transpose(0, 4, 3, 1, 2, 5))
        w2_c = w2_c.reshape(N_PASSES, O // P, P, E_PER_PASS * KT, P).astype(bf16)
        m = {
            "xt": XT,
            "w1": w1_c,
            "w2t": w2_c,
            "gw": np.ascontiguousarray(
                gate_W[:, perm].reshape(KT, P, E).transpose(1, 0, 2)).astype(bf16),
        }
        if _CFG["fp8_kp"]:
            m["xt8"] = XT8
            # [e, P, NKP8, 2, H]
            m["w18"] = np.ascontiguousarray(
                W1[own][:, :KF8, :].reshape(E_PER_CORE, NKP8, 2, P, H)
                .transpose(0, 3, 1, 2, 4)).astype(ml_dtypes.float8_e4m3)
        if use_gb:
            m["gb"] = np.ascontiguousarray(gate_b[perm])
        if use_b1:
            m["b1"] = np.ascontiguousarray(b1[own])
        if use_lng:
            m["lng"] = np.ascontiguousarray(ln_g[own])
        if use_lnb:
            m["lnb"] = np.ascontiguousarray(ln_b[own])
        in_maps.append(m)

    global _LAST_IN_MAPS
    _LAST_IN_MAPS = in_maps
    res = run_bass_kernel_spmd(nc, in_maps, core_ids=list(range(N_CORES)))

    acc = np.zeros((O, B), dtype=np.float32)
    for c in range(N_CORES):
        part = res.results[c]["outp"]
        acc += part[0].astype(np.float32)
        acc += part[1].astype(np.float32)
    out = np.ascontiguousarray(acc.T)
    if np.any(b2 != 0.0):
        # per-expert output bias enters as gate @ b2 ([B,E] @ [E,O])
        gate = X @ gate_W + gate_b[None, :]
        gate -= gate.max(axis=1, keepdims=True)
        np.exp(gate, out=gate)
        gate /= gate.sum(axis=1, keepdims=True)
        out += gate @ b2
    return np.ascontiguousarray(out, dtype=np.float32)


if __name__ == "__main__":
    rng = np.random.default_rng(0)
    s = 0.02
    inputs = {
        "pooled_output": rng.standard_normal((B, H), dtype=np.float32),
        "gate_W": rng.standard_normal((H, E), dtype=np.float32) * s,
        "gate_b": np.zeros((E,), np.float32),
        "W1": rng.standard_normal((E, H, H), dtype=np.float32) * s,
        "b1": np.zeros((E, H), np.float32),
        "ln_g": np.ones((E, H), np.float32),
        "ln_b": np.zeros((E, H), np.float32),
        "W2": rng.standard_normal((E, H, O), dtype=np.float32) * s,
        "b2": np.zeros((E, O), np.float32),
    }
    out = kernel(**inputs)
    print("out", out.shape, out.dtype, np.abs(out).max())
